# revision 2
# baseline (speedup 1.0000x reference)
"""GAT layer kernel v2 for Trainium2 (8 NeuronCores, Bass/Tile).

Strategy: destination rows sharded across 8 cores (12500 each). Phase A
computes h = x@W+b, wh1 = h@a[:64], wh2 = h@a[64:] for own rows; packs a
table of 768B-stride rows [feats(4 nodes) | wh2 x4] (520B used); all-gathers
the table (19.9MB); phase B gathers 520B per edge with dma_gather
(idx = col>>2, int16), computes the segment softmax with PE group-sum +
expand, builds masked-attention weights sam4 on DVE (tensor_scalar with
per-partition scalar), and aggregates with 64 small PE matmuls per block
into psum[8, 16, 64].

Layout (dest-major): block b = 128 dests; chunk c holds dests 8c..8c+7;
edge k of dest d sits at partition 16*(d%8)+k of chunk d//8. Per-edge
scalars live at [partition, chunk]; the dest-group (16-partition) sums go
through PE with constant selectors.
"""

import sys
import types

import numpy as np

sys.path.insert(0, "/opt/trn_rl_repo")

N = 100000
DEG = 16
E = N * DEG
IN_F = 128
OUT_F = 64
ALPHA = 0.2
EPS = 1e-12

NCORES = 8
NLOC = N // NCORES              # 12500 dest rows per core
P = 128
NBLK = (NLOC + P - 1) // P      # 98 blocks of 128 dests
LAST_VALID = NLOC - (NBLK - 1) * P  # 84 dests in last block

PACK = 4
RS = 384                        # table row stride in f16 elems (768B)
FETCH = 260                     # f16 elems fetched per edge (520B)
TROWS = N // PACK               # 25000 table rows
TROWS_LOC = NLOC // PACK        # 3125 local table rows
GIDX = 1024                     # indices per dma_gather call
CALLS = 2048 // GIDX            # gather calls per block


def _install_ntff_shim():
    if "antenv.axon_hooks" in sys.modules:
        return
    try:
        from trn_agent_boot.trn_boot import _ntff_profile_via_ctypes

        hook = _ntff_profile_via_ctypes("/opt/axon/libaxon_pjrt.so")
    except Exception:
        hook = None
    mod = types.ModuleType("antenv.axon_hooks")
    mod.get_axon_ntff_profile_hook = lambda: hook
    mod.set_axon_ntff_profile_hook = lambda h: None
    sys.modules["antenv.axon_hooks"] = mod


def _install_dma_gather_patch():
    """Relax bass's elem_size%256 assert (ucode needs it only for transpose)."""
    import inspect
    import textwrap

    import concourse.bass as bass

    if getattr(bass.BassGpSimd.dma_gather, "_gat_patched", False):
        return
    src = textwrap.dedent(inspect.getsource(bass.BassGpSimd.dma_gather))
    old = """    assert (
        elem_size_bytes > 0 and elem_size_bytes % 256 == 0
    )  # transpose restriction"""
    new = """    assert elem_size_bytes > 0
    if transpose:
        assert elem_size_bytes % 256 == 0"""
    assert old in src, "dma_gather source changed; patch needs updating"
    src = src.replace(old, new)
    g = dict(bass.__dict__)
    exec(src, g)
    g["dma_gather"]._gat_patched = True
    bass.BassGpSimd.dma_gather = g["dma_gather"]


_PROGRAM_CACHE = {}


def build_program():
    _install_ntff_shim()
    _install_dma_gather_patch()
    import concourse.bacc as bacc
    import concourse.bass as bass
    import concourse.tile as tile
    from concourse import mybir

    f32 = mybir.dt.float32
    f16 = mybir.dt.float16
    i16 = mybir.dt.int16

    nc = bacc.Bacc(
        "TRN2",
        num_devices=NCORES,
        num_swdge_queues=4,
        detect_race_conditions=False,
    )

    inT = nc.dram_tensor("inT", [IN_F, NLOC], f32, kind="ExternalInput")
    W_in = nc.dram_tensor("W_in", [IN_F, OUT_F], f32, kind="ExternalInput")
    a2_in = nc.dram_tensor("a2_in", [OUT_F, 2], f32, kind="ExternalInput")
    bias_in = nc.dram_tensor("bias_in", [OUT_F], f32, kind="ExternalInput")
    pk_in = nc.dram_tensor("pk_in", [NBLK, P, 704], i16,
                           kind="ExternalInput")
    e8c_in = nc.dram_tensor("e8c_in", [8, P], f32, kind="ExternalInput")
    s16s_in = nc.dram_tensor("s16s_in", [P, 8], f32, kind="ExternalInput")

    out_d = nc.dram_tensor("out_d", [NLOC, OUT_F], f32, kind="ExternalOutput")

    with tile.TileContext(nc) as tc:
        with tc.tile_pool(name="dram", bufs=1, space="DRAM") as dpool:
            h4_loc = dpool.tile([TROWS_LOC, RS], f16)
            h4 = dpool.tile([TROWS, RS], f16, addr_space="Shared")
            wh1b_d = dpool.tile([8, NBLK * 16], f32)

            with tc.tile_pool(name="const", bufs=1) as cpool:
                w_sb = cpool.tile([IN_F, OUT_F], f32)
                nc.sync.dma_start(out=w_sb[:], in_=W_in[:])
                a2_sb = cpool.tile([OUT_F, 2], f32)
                nc.sync.dma_start(out=a2_sb[:], in_=a2_in[:])
                bias_col = cpool.tile([OUT_F, 1], f32)
                nc.sync.dma_start(out=bias_col[:], in_=bias_in[:, None])
                e8c = cpool.tile([8, P], f32)
                nc.sync.dma_start(out=e8c[:], in_=e8c_in[:])
                s16s = cpool.tile([P, 8], f32)
                nc.sync.dma_start(out=s16s[:], in_=s16s_in[:])
                bias_rep = cpool.tile([P, OUT_F], f32)
                bin_ap = bias_in[:]
                nc.sync.dma_start(
                    out=bias_rep[:],
                    in_=bass.AP(bin_ap.tensor, bin_ap.offset,
                                [[0, P], [1, OUT_F]]),
                )
                wh1_all = cpool.tile([P, NBLK], f32)
                nc.vector.memset(wh1_all[:], 0.0)

                # prologue: wa2 = W @ a2 (contract over OUT_F), cv = a2^T bias
                with tc.tile_pool(name="pa", bufs=1, space="PSUM") as pp0, \
                        tc.tile_pool(name="sa", bufs=1) as sp0:
                    idp = sp0.tile([P, P], f32)
                    from concourse.masks import make_identity

                    make_identity(nc, idp[:])
                    wt_ps = pp0.tile([P, P], f32, space="PSUM")
                    nc.tensor.transpose(out=wt_ps[:OUT_F, :IN_F], in_=w_sb[:],
                                        identity=idp[:])
                    wt_sb = sp0.tile([OUT_F, IN_F], f32)
                    nc.vector.tensor_copy(out=wt_sb[:], in_=wt_ps[:OUT_F, :IN_F])
                    wa2_ps = pp0.tile([IN_F, 2], f32, space="PSUM")
                    nc.tensor.matmul(out=wa2_ps[:], lhsT=wt_sb[:], rhs=a2_sb[:])
                    wa2_sb = cpool.tile([IN_F, 2], f32)
                    nc.vector.tensor_copy(out=wa2_sb[:], in_=wa2_ps[:])
                    ab_ps = pp0.tile([2, 1], f32, space="PSUM")
                    nc.tensor.matmul(out=ab_ps[:], lhsT=a2_sb[:], rhs=bias_col[:])
                    ab_sb = sp0.tile([2, 1], f32)
                    nc.vector.tensor_copy(out=ab_sb[:], in_=ab_ps[:])
                    ab_dram = dpool.tile([2], f32)
                    nc.sync.dma_start(
                        out=bass.AP(ab_dram[:].tensor, ab_dram[:].offset,
                                    [[1, 2], [1, 1]]),
                        in_=ab_sb[:],
                    )
                    cv2_rep = cpool.tile([P, 1], f32)
                    cv1_rep = cpool.tile([P, 1], f32)
                    nc.sync.dma_start(
                        out=cv2_rep[:],
                        in_=bass.AP(ab_dram[:].tensor, ab_dram[:].offset,
                                    [[0, P], [1, 1]]),
                    )
                    nc.sync.dma_start(
                        out=cv1_rep[:],
                        in_=bass.AP(ab_dram[:].tensor, ab_dram[:].offset + 1,
                                    [[0, P], [1, 1]]),
                    )

                # ---------------- phase A ----------------
                with tc.tile_pool(name="pha_s", bufs=1) as spA, \
                        tc.tile_pool(name="pha_ps", bufs=4, space="PSUM") as ppA, \
                        tc.tile_pool(name="pha_w", bufs=4) as wpA:
                    bf16 = mybir.dt.bfloat16
                    w16_all = spA.tile([P, NBLK], f16)
                    nc.vector.memset(w16_all[:], 0.0)
                    inT_sb = spA.tile([IN_F, NLOC], f32)
                    NCH = NLOC // 4
                    for ch in range(4):
                        nc.sync.dma_start(
                            out=inT_sb[:, ch * NCH:(ch + 1) * NCH],
                            in_=inT[:, ch * NCH:(ch + 1) * NCH])
                    inT_bf = spA.tile([IN_F, NLOC], bf16)
                    for ch in range(4):
                        nc.vector.tensor_copy(
                            out=inT_bf[:, ch * NCH:(ch + 1) * NCH],
                            in_=inT_sb[:, ch * NCH:(ch + 1) * NCH])
                    w_bf = spA.tile([IN_F, OUT_F], bf16)
                    nc.vector.tensor_copy(out=w_bf[:], in_=w_sb[:])
                    wa2_bf = spA.tile([IN_F, 2], bf16)
                    nc.vector.tensor_copy(out=wa2_bf[:], in_=wa2_sb[:])

                    for t in range(NBLK):
                        r0 = t * P
                        rows = min(P, NLOC - r0)
                        lhsT = inT_bf[:, r0:r0 + rows]
                        h_ps = ppA.tile([P, OUT_F], f32, tag="h_ps")
                        nc.tensor.matmul(out=h_ps[:rows, :], lhsT=lhsT, rhs=w_bf[:])
                        whT_ps = ppA.tile([P, 2], f32, tag="whT_ps")
                        nc.tensor.matmul(out=whT_ps[:rows, :], lhsT=lhsT,
                                         rhs=wa2_bf[:])
                        h16 = wpA.tile([P, OUT_F], f16, tag="h16")
                        nc.vector.tensor_add(out=h16[:rows, :], in0=h_ps[:rows, :],
                                             in1=bias_rep[:rows, :])
                        nc.vector.tensor_add(out=w16_all[:rows, t:t + 1],
                                             in0=whT_ps[:rows, 0:1],
                                             in1=cv2_rep[:rows, :])
                        nc.vector.tensor_add(out=wh1_all[:rows, t:t + 1],
                                             in0=whT_ps[:rows, 1:2],
                                             in1=cv1_rep[:rows, :])
                        trow = (r0 // PACK)
                        nrows = rows // PACK
                        eng = nc.sync if t % 2 == 0 else nc.scalar
                        eng.dma_start(
                            out=bass.AP(h4_loc[:].tensor,
                                        h4_loc[:].offset + trow * RS,
                                        [[RS, nrows], [OUT_F, PACK], [1, OUT_F]]),
                            in_=h16[:rows, :],
                        )
                        nc.gpsimd.dma_start(
                            out=bass.AP(h4_loc[:].tensor,
                                        h4_loc[:].offset + trow * RS + 4 * OUT_F,
                                        [[RS, nrows], [1, PACK]]),
                            in_=w16_all[:rows, t:t + 1],
                        )
                    # one flush for wh1 in [8, NBLK*16] block layout:
                    # (p, t) -> [p%8, 16t + p//8]
                    nc.sync.dma_start(
                        out=bass.AP(wh1b_d[:].tensor, wh1b_d[:].offset,
                                    [[1, 16], [NBLK * 16, 8], [16, NBLK]]),
                        in_=wh1_all[:],
                    )

                # ---------------- all-gather ----------------
                nc.gpsimd.collective_compute(
                    "AllGather",
                    mybir.AluOpType.bypass,
                    replica_groups=[list(range(NCORES))],
                    ins=[h4_loc.opt()],
                    outs=[h4.opt()],
                )

                # ---------------- phase B ----------------
                with tc.tile_pool(name="phb", bufs=4) as bp, \
                        tc.tile_pool(name="phb_ps", bufs=3, space="PSUM") as bpp, \
                        tc.tile_pool(name="phb_po", bufs=2, space="PSUM") as bpo, \
                        tc.tile_pool(name="phb_g", bufs=4) as gp:
                    PF = 2  # gather prefetch depth (blocks)
                    nch = 16 // CALLS
                    inflight = []
                    for bb in range(NBLK + PF):
                        if bb < NBLK:
                            pk = bp.tile([P, 704], i16, tag="pk")
                            nc.sync.dma_start(out=pk[:], in_=pk_in[bb])
                            wh1b = bp.tile([8, 16], f32, tag="wh1b")
                            nc.scalar.dma_start(
                                out=wh1b[:],
                                in_=bass.AP(wh1b_d[:].tensor,
                                            wh1b_d[:].offset + 16 * bb,
                                            [[NBLK * 16, 8], [1, 16]]))
                            gt_n = []
                            for hf in range(CALLS):
                                gh = gp.tile([P, nch, FETCH], f16, tag=f"g{hf}")
                                nc.gpsimd.dma_gather(
                                    out_ap=gh[:],
                                    in_ap=h4[:, 0:FETCH],
                                    idxs_ap=pk[:, hf * 64:(hf + 1) * 64],
                                    num_idxs=GIDX,
                                    num_idxs_reg=GIDX,
                                    elem_size=FETCH,
                                    elem_step=RS,
                                    queue_num=(CALLS * bb + hf) % 4,
                                )
                                gt_n.append(gh)
                            inflight.append((bb, pk, wh1b, gt_n))
                        if bb < PF:
                            continue
                        b, pk, wh1b, gt = inflight.pop(0)
                        soh4 = pk[:, 128:640].bitcast(f16)   # [P, 512]
                        oh4 = pk[:, 640:704].bitcast(f16)    # [P, 64]

                        # wh2 per edge: tails at [c, 256:260], mask by oh4
                        e64 = bp.tile([P, 64], f16, tag="e64")
                        for hf in range(CALLS):
                            gh = gt[hf]
                            nc.vector.tensor_copy(
                                out=e64[:, hf * 32:(hf + 1) * 32],
                                in_=bass.AP(gh[:].tensor, gh[:].offset + 4 * OUT_F,
                                            [list(gh[:].ap[0]), [FETCH, nch],
                                             [1, 4]]))
                        wsel = bp.tile([P, 64], f16, tag="wsel")
                        nc.vector.tensor_mul(out=wsel[:], in0=e64[:], in1=oh4)
                        wh2e = bp.tile([P, 16], f32, tag="wh2e")
                        nc.vector.reduce_sum(
                            out=wh2e[:],
                            in_=wsel[:].rearrange("p (a b) -> p a b", a=16),
                            axis=mybir.AxisListType.X)

                        # wh1e[p, c] = wh1[8c + p//16] via PE expansion
                        sps = bpp.tile([P, 48], f32, space="PSUM", tag="sps")
                        nc.tensor.matmul(out=sps[:, 0:16], lhsT=e8c[:],
                                         rhs=wh1b[:])
                        s = bp.tile([P, 16], f32, tag="s")
                        nc.vector.tensor_add(out=s[:], in0=wh2e[:],
                                             in1=sps[:, 0:16])
                        # exp(leakyrelu(s)) == max(exp(s), exp(alpha*s))
                        e1 = bp.tile([P, 16], f32, tag="e1")
                        nc.scalar.activation(out=e1[:], in_=s[:],
                                             func=mybir.ActivationFunctionType.Exp)
                        e2 = bp.tile([P, 16], f32, tag="e2")
                        nc.scalar.activation(out=e2[:], in_=s[:],
                                             func=mybir.ActivationFunctionType.Exp,
                                             scale=ALPHA)
                        ex = bp.tile([P, 16], f32, tag="ex")
                        nc.vector.tensor_tensor(out=ex[:], in0=e1[:], in1=e2[:],
                                                op=mybir.AluOpType.max)

                        nc.tensor.matmul(out=sps[0:8, 16:32], lhsT=s16s[:],
                                         rhs=ex[:])
                        rden = bp.tile([8, 16], f32, tag="rden")
                        nc.vector.reciprocal(out=rden[:], in_=sps[0:8, 16:32])
                        nc.tensor.matmul(out=sps[:, 32:48], lhsT=e8c[:],
                                         rhs=rden[:])
                        attn = bp.tile([P, 16], f16, tag="attn")
                        nc.vector.tensor_mul(out=attn[:], in0=ex[:],
                                             in1=sps[:, 32:48])

                        # sam4[p, (c, m, j)] = soh4 * attn[p, c] (rep 32x)
                        attnrep = bp.tile([P, 512], f16, tag="attnrep")
                        nc.vector.tensor_copy(
                            out=attnrep[:],
                            in_=bass.AP(attn[:].tensor, attn[:].offset,
                                        [list(attn[:].ap[0]), [1, 16], [0, 32]]))
                        sam4 = bp.tile([P, 512], f16, tag="sam4")
                        nc.vector.tensor_mul(out=sam4[:], in0=soh4,
                                             in1=attnrep[:])

                        # aggregation: psum[8, c, f] = sum_m sam4_cm^T @ g_cm
                        outp = bpo.tile([8, 16, OUT_F], f32, space="PSUM",
                                        tag="outp")
                        for c in range(16):
                            gh = gt[c // nch]
                            cc = c % nch
                            for m in range(PACK):
                                nc.tensor.matmul(
                                    out=outp[:, c, :],
                                    lhsT=sam4[:, c * 32 + m * 8:c * 32 + (m + 1) * 8],
                                    rhs=gh[:, cc, m * OUT_F:(m + 1) * OUT_F],
                                    start=(m == 0),
                                    stop=(m == PACK - 1),
                                )
                        osb = bp.tile([8, 16, OUT_F], f32, tag="osb")
                        nc.scalar.copy(
                            out=osb[:].rearrange("p a b -> p (a b)"),
                            in_=outp[:].rearrange("p a b -> p (a b)"))
                        rows = P if b < NBLK - 1 else LAST_VALID
                        kfull = rows // 8
                        crem = rows - kfull * 8
                        nc.sync.dma_start(
                            out=bass.AP(
                                out_d[:].tensor,
                                out_d[:].offset + b * P * OUT_F,
                                [[OUT_F, 8], [8 * OUT_F, kfull], [1, OUT_F]]),
                            in_=osb[:, 0:kfull, :],
                        )
                        if crem:
                            nc.sync.dma_start(
                                out=bass.AP(
                                    out_d[:].tensor,
                                    out_d[:].offset + (b * P + kfull * 8) * OUT_F,
                                    [[OUT_F, crem], [8 * OUT_F, 1], [1, OUT_F]]),
                                in_=osb[0:crem, kfull:kfull + 1, :],
                            )

    nc.compile()
    return nc


def _host_prep(input_h, W, a, bias, indices):
    """Build the 8 per-core in_maps. Index-side layout prep only."""
    idx = np.ascontiguousarray(indices.astype(np.int64))
    a2 = np.concatenate([a[OUT_F:], a[:OUT_F]], axis=1).astype(np.float32)

    # constant selector matrices
    e8c = np.zeros((8, P), dtype=np.float32)
    for p in range(P):
        e8c[p // 16, p] = 1.0
    s16s = np.ascontiguousarray(e8c.T)

    # position mapping: position J (0..2047) in block -> partition J%128,
    # chunk J//128; edge (d, k) of dest d at partition 16*(d%8)+k, chunk d//8
    pp_, cc_ = np.meshgrid(np.arange(P), np.arange(16), indexing="ij")
    dloc = 8 * cc_ + pp_ // 16            # [p, c] dest-local id
    kk = pp_ % 16                          # [p, c] edge index within dest

    in_maps = []
    for core in range(NCORES):
        r0 = core * NLOC
        inT = np.ascontiguousarray(input_h[r0:r0 + NLOC].T)
        ecols = idx[r0 * DEG:(r0 + NLOC) * DEG].reshape(NLOC, DEG)
        ecols = np.pad(ecols, ((0, NBLK * P - NLOC), (0, 0)))  # [12544, 16]

        # per block: cols at [p, c] positions
        colpc = ecols.reshape(NBLK, P, DEG)[
            np.arange(NBLK)[:, None, None], dloc[None], kk[None]]  # [b, p, c]
        colq = (colpc >> 2).astype(np.int16)   # table row
        colm = (colpc & 3).astype(np.int64)    # slot

        # idx wrap: position J = 16*i + q -> partition q (+16 rep), free i
        # J = 128*c + p  =>  per call hf: positions J in [1024*hf, 1024*(hf+1))
        Jcol = colq.transpose(0, 2, 1).reshape(NBLK, 2048)  # J-ordered
        cw = Jcol.reshape(NBLK, CALLS, GIDX // 16, 16)
        idx16w_a = np.ascontiguousarray(cw.transpose(0, 1, 3, 2))
        idx16w_a = np.tile(idx16w_a, (1, 1, 8, 1))  # [b, call, 128, GIDX//16]

        # oh4[p, (c, m)] mask of the edge's slot; zero for invalid dests
        valid = np.ones((NBLK, P, 16), dtype=bool)
        vd = dloc[None] + np.arange(NBLK)[:, None, None] * P
        valid &= vd < NLOC
        oh4 = np.zeros((NBLK, P, 16, 4), dtype=np.float16)
        bb2, pp2, cc2 = np.meshgrid(np.arange(NBLK), np.arange(P),
                                    np.arange(16), indexing="ij")
        oh4[bb2, pp2, cc2, colm[bb2, pp2, cc2]] = 1.0
        oh4 *= valid[..., None]


        # soh4[p, c, m, j] = oh4 * (j == p//16)
        e8p = np.eye(8, dtype=np.float16)[np.arange(P) // 16]  # [P, 8]
        soh4 = oh4[..., None] * e8p[None, :, None, None, :]
        # pack [offs i16 128 | soh4 f16 512 | oh4 f16 64] per partition
        pk = np.empty((NBLK, P, 704), dtype=np.int16)
        offs_pm = idx16w_a.reshape(NBLK, CALLS, P, GIDX // 16)
        pk[:, :, 0:128] = offs_pm.transpose(0, 2, 1, 3).reshape(NBLK, P, 128)
        pk[:, :, 128:640] = soh4.reshape(NBLK, P, 512).view(np.int16)
        pk[:, :, 640:704] = oh4.reshape(NBLK, P, 64).view(np.int16)
        in_maps.append({
            "inT": inT.astype(np.float32),
            "W_in": np.asarray(W, dtype=np.float32),
            "a2_in": a2,
            "bias_in": np.asarray(bias, dtype=np.float32),
            "pk_in": pk,
            "e8c_in": e8c,
            "s16s_in": s16s,
        })
    return in_maps


def _reference_numpy(input_h, W, a, bias, indptr, indices):
    h = input_h.astype(np.float64) @ W.astype(np.float64) + bias.astype(np.float64)
    deg = np.diff(indptr.astype(np.int64))
    row = np.repeat(np.arange(N, dtype=np.int64), deg)
    e_cnt = indices.shape[0]
    if row.shape[0] < e_cnt:
        pad_val = row[-1] if row.shape[0] else 0
        row = np.pad(row, (0, e_cnt - row.shape[0]), constant_values=pad_val)
    row = row[:e_cnt]
    col = indices.astype(np.int64)
    a_src = a[:OUT_F, 0].astype(np.float64)
    a_dst = a[OUT_F:, 0].astype(np.float64)
    wh1 = h @ a_src
    wh2 = h @ a_dst
    e = wh1[row] + wh2[col]
    e = np.where(e >= 0, e, ALPHA * e)
    emax = np.full(N, -np.inf)
    np.maximum.at(emax, row, e)
    ex = np.exp(e - emax[row])
    den = np.zeros(N)
    np.add.at(den, row, ex)
    attn = ex / (den[row] + EPS)
    out = np.zeros((N, OUT_F))
    np.add.at(out, row, attn[:, None] * h[col])
    return out.astype(np.float32)


def kernel(input_h, W, a, bias, indptr, indices):
    input_h = np.asarray(input_h, dtype=np.float32)
    W = np.asarray(W, dtype=np.float32)
    a = np.asarray(a, dtype=np.float32)
    bias = np.asarray(bias, dtype=np.float32)
    indptr = np.asarray(indptr)
    indices_np = np.asarray(indices)

    expected_indptr = np.arange(N + 1, dtype=np.int64) * DEG
    if (
        indptr.shape[0] != N + 1
        or indices_np.shape[0] != E
        or not np.array_equal(indptr.astype(np.int64), expected_indptr)
    ):
        return _reference_numpy(input_h, W, a, bias, indptr, indices_np)

    _install_ntff_shim()
    _install_dma_gather_patch()
    from concourse.bass_utils import run_bass_kernel_spmd

    key = "gat"
    if key not in _PROGRAM_CACHE:
        _PROGRAM_CACHE[key] = build_program()
    nc = _PROGRAM_CACHE[key]

    in_maps = _host_prep(input_h, W, a, bias, indices_np)
    res = run_bass_kernel_spmd(nc, in_maps, core_ids=list(range(NCORES)))
    out = np.concatenate([res.results[c]["out_d"] for c in range(NCORES)], axis=0)
    return out.astype(np.float32)


if __name__ == "__main__":
    pass
